# revision 12
# baseline (speedup 1.0000x reference)
"""GraphSAGE (2x SAGEConv mean-aggr + log_softmax) on 8 Trainium2 NeuronCores.

v2 strategy (graph/data parallel):
  - Nodes sharded into 8 ranges of 12544; within each core, 128-node dst
    blocks are HOST-RELABELED into descending-edge-count order so the
    cross-core max tile schedule stays tight while the SPMD program is
    uniform (host un-permutes the output).
  - All features bf16 on device (4x PE, 2x DVE, ~2x gather DMA vs f32).
  - Edges routed to dst core, grouped per (dst block, 25088-row src chunk);
    per (7-block group, chunk) dma_gather with ~4-6k indices, spread over
    4 SWDGE queues (desc-gen parallelizes across Q7 core pairs).
  - Aggregation: one-hot S (DVE is_equal, bf16) x gathered messages on PE;
    layer-1 runs feature-major (aggT = msg.T @ S) so the dense chain
    h = relu(mean@W1_l + x@W1_r + b1), [z2|r2] = h@[W2_l|W2_r] needs no
    on-device transposes (weights consumed as stored).
  - deg^-1 applied via a host-precomputed broadcast plane (feature-major)
    or per-partition scalars (node-major layer 2).
  - z2 stored duplicated [z2|z2] bf16 (256 B rows: dma_gather minimum),
    exchanged with an in-kernel AllGather, then layer 2 aggregates
    node-major and finishes log_softmax with Exp(accum_out)/Ln on ACT.
"""

import sys

import numpy as np

sys.path.insert(0, "/opt/trn_rl_repo")

P = 128
D = 128
DO = 64
CORES = 8
GRP = 5
PAD_LDST = 240.0


def _prep(x, edge_index, n_nodes, shard, chunk, grp):
    npad = shard * CORES
    nblk = shard // P
    n_chunks = -(-npad // chunk)
    src = np.asarray(edge_index[0], dtype=np.int64)
    dst = np.asarray(edge_index[1], dtype=np.int64)

    # ---- host node relabel: per-core blocks sorted by edge count ----
    bc = np.bincount(dst // P, minlength=CORES * nblk).reshape(CORES, nblk)
    perm = np.empty(npad, dtype=np.int64)  # old id -> new id
    for c in range(CORES):
        order = np.argsort(-bc[c], kind="stable")  # rank -> old local block
        inv = np.empty(nblk, np.int64)
        inv[order] = np.arange(nblk)
        j = np.arange(shard)
        perm[c * shard + j] = c * shard + inv[j // P] * P + j % P
    src = perm[src]
    dst = perm[dst]
    x_pad = np.zeros((npad, D), np.float32)
    x_pad[perm[:n_nodes]] = np.asarray(x, np.float32)

    deg = np.bincount(dst, minlength=npad).astype(np.float32)
    deginv = (1.0 / np.maximum(deg, 1.0)).astype(np.float32)

    core = dst // shard
    lb = (dst % shard) // P  # rank-block
    ch = src // chunk

    cnt = np.zeros((CORES, nblk, n_chunks), np.int64)
    np.add.at(cnt, (core, lb, ch), 1)
    t_run = -(-cnt.max(axis=0) // P)  # [nblk, n_chunks]
    t_run[:, 0] = np.maximum(t_run[:, 0], 1)
    TS = t_run.sum(axis=1)  # tiles per block (S layout)
    nt = int(TS.sum())
    tmax = int(TS.max())

    # S/ldst column layout: block-major, chunks consecutive within block
    col_base_blk = np.concatenate([[0], np.cumsum(TS)])  # [nblk+1]
    ctb = np.concatenate(
        [np.zeros((nblk, 1), np.int64), np.cumsum(t_run, axis=1)[:, :-1]], axis=1
    )  # chunk tile base within block

    # group structure
    n_groups = -(-nblk // grp)
    G_T = np.zeros((n_groups, n_chunks), np.int64)  # tiles per (group, chunk)
    sec_base = np.zeros((nblk, n_chunks), np.int64)  # block tile base in section
    for g in range(n_groups):
        r0, r1 = g * grp, min((g + 1) * grp, nblk)
        for c_ in range(n_chunks):
            acc = 0
            for r in range(r0, r1):
                sec_base[r, c_] = acc
                acc += t_run[r, c_]
            G_T[g, c_] = acc
    # idx stream col bases (x8 cols per tile): gather order (g, ch)
    ig_tiles = G_T.ravel()
    idx_tile_base = np.concatenate([[0], np.cumsum(ig_tiles)]).reshape(-1)[:-1]
    idx_tile_base = idx_tile_base.reshape(n_groups, n_chunks)
    assert ig_tiles.sum() == nt

    order = np.lexsort((dst, ch, lb, core))
    src_s, dst_s, lb_s, ch_s, core_s = (
        src[order], dst[order], lb[order], ch[order], core[order]
    )

    idx16 = np.zeros((CORES, 16, nt * 8), np.int16)
    ldst = np.full((CORES, P, nt), PAD_LDST, np.float32)
    for c in range(CORES):
        m = core_s == c
        lbc, chc, d_c, s_c = lb_s[m], ch_s[m], dst_s[m], src_s[m]
        rid = lbc * n_chunks + chc
        starts = np.concatenate(
            [[0], np.cumsum(np.bincount(rid, minlength=nblk * n_chunks))]
        )
        pos = np.arange(len(d_c)) - starts[rid]
        colt = col_base_blk[lbc] + ctb[lbc, chc] + pos // P
        ldst[c, pos % P, colt] = (d_c % P).astype(np.float32)
        g = lbc // grp
        slot = (sec_base[lbc, chc] + pos // P) * P + pos % P
        icol = idx_tile_base[g, chc] * 8 + slot // 16
        idx16[c, slot % 16, icol] = (s_c - chc * chunk).astype(np.int16)
    idx16 = np.tile(idx16, (1, 8, 1))  # replicate 16 -> 128 partitions

    return dict(
        x_pad=x_pad, perm=perm, deginv=deginv, idx16=idx16, ldst=ldst,
        t_run=t_run, TS=TS, nt=nt, tmax=tmax, nblk=nblk, npad=npad,
        n_chunks=n_chunks, n_groups=n_groups, col_base_blk=col_base_blk,
        ctb=ctb, G_T=G_T, sec_base=sec_base, idx_tile_base=idx_tile_base,
    )


def _build(meta, shard, chunk, grp):
    import concourse.mybir as mybir
    import concourse.tile as tile
    from concourse.bacc import Bacc

    f32 = mybir.dt.float32
    bf16 = mybir.dt.bfloat16
    i16 = mybir.dt.int16
    Alu = mybir.AluOpType
    Act = mybir.ActivationFunctionType
    X = mybir.AxisListType.X

    nblk = meta["nblk"]
    npad = meta["npad"]
    nt = meta["nt"]
    tmax = meta["tmax"]
    n_chunks = meta["n_chunks"]
    n_groups = meta["n_groups"]
    t_run = meta["t_run"]
    col_base_blk = meta["col_base_blk"]
    ctb = meta["ctb"]
    G_T = meta["G_T"]
    sec_base = meta["sec_base"]
    idx_tile_base = meta["idx_tile_base"]
    TS = meta["TS"]
    GTmax = [int(G_T[:, c_].max()) for c_ in range(n_chunks)]

    nc = Bacc(num_swdge_queues=4)
    # ---- kernel I/O ----
    x_pad_d = nc.dram_tensor("x_pad", [npad, D], bf16, kind="ExternalInput")
    xt_d = nc.dram_tensor("xt", [P, shard], bf16, kind="ExternalInput")
    dgb_d = nc.dram_tensor("dgb", [P, shard], bf16, kind="ExternalInput")
    idx_d = nc.dram_tensor("idx16", [P, nt * 8], i16, kind="ExternalInput")
    ldst_d = nc.dram_tensor("ldst", [P, nt], bf16, kind="ExternalInput")
    deginv_d = nc.dram_tensor("deginv", [P, nblk], f32, kind="ExternalInput")
    w1l_d = nc.dram_tensor("w1l", [D, D], bf16, kind="ExternalInput")
    w1r_d = nc.dram_tensor("w1r", [D, D], bf16, kind="ExternalInput")
    w2cat_d = nc.dram_tensor("w2cat", [D, 2 * DO], bf16, kind="ExternalInput")
    b1c_d = nc.dram_tensor("b1c", [P, 1], f32, kind="ExternalInput")
    b2b_d = nc.dram_tensor("b2b", [P, DO], f32, kind="ExternalInput")
    iota_d = nc.dram_tensor("iota", [P, P], bf16, kind="ExternalInput")
    out_d = nc.dram_tensor("out", [shard, DO], f32, kind="ExternalOutput")

    with tile.TileContext(nc) as tc:
        with (
            tc.tile_pool(name="const", bufs=1) as cp,
            tc.tile_pool(name="stream", bufs=2) as sp2,
            tc.tile_pool(name="msg", bufs=3) as mp,
            tc.tile_pool(name="sel", bufs=2) as sep,
            tc.tile_pool(name="work", bufs=3) as wp,
            tc.tile_pool(name="pipe", bufs=grp + 1) as op8,
            tc.tile_pool(name="dram", bufs=1, space="DRAM") as dp,
            tc.tile_pool(name="psum", bufs=2, space="PSUM") as pp,
        ):
            z2_own_t = dp.tile([shard, 2 * DO], bf16)
            z2_full_t = dp.tile([npad, 2 * DO], bf16)
            w1l = cp.tile_from(w1l_d[:, :])
            w1r = cp.tile_from(w1r_d[:, :])
            w2cat = cp.tile_from(w2cat_d[:, :])
            b1c = cp.tile_from(b1c_d[:, :])
            b2b = cp.tile_from(b2b_d[:, :])
            iota = cp.tile_from(iota_d[:, :])
            deginv = cp.tile_from(deginv_d[:, :])
            idx = cp.tile_from(idx_d[:, :])
            ldst = cp.tile_from(ldst_d[:, :])
            r2_all = cp.tile([P, nblk * DO], bf16)

            def build_sel(r):
                t = int(TS[r])
                c0 = int(col_base_blk[r])
                s_t = sep.tile([P, tmax * P], bf16, tag="S")
                l0 = ldst[:, c0 : c0 + t][:, :, None].to_broadcast([P, t, P])
                i0 = iota[:, :][:, None, :].to_broadcast([P, t, P])
                nc.vector.tensor_tensor(
                    out=s_t[:, : t * P].rearrange("p (t s) -> p t s", s=P),
                    in0=l0,
                    in1=i0,
                    op=Alu.is_equal,
                )
                return s_t

            def gathers(g, src_view, qoff):
                """Launch the n_chunks gathers for group g; returns msg tiles."""
                msgs = []
                for c_ in range(n_chunks):
                    gt = int(G_T[g][c_])
                    m = mp.tile([P, GTmax[c_] * D], bf16, tag=f"m{c_}")
                    nidx = gt * P
                    nc.gpsimd.dma_gather(
                        out_ap=m[:, : gt * D].rearrange("p (t d) -> p t d", d=D),
                        in_ap=src_view(c_),
                        idxs_ap=idx[
                            :,
                            int(idx_tile_base[g][c_]) * 8 : (
                                int(idx_tile_base[g][c_]) + gt
                            )
                            * 8,
                        ],
                        num_idxs=nidx,
                        num_idxs_reg=nidx,
                        elem_size=D,
                        elem_step=D,
                        single_packet=False,
                        queue_num=(g + c_ + qoff) % 4,
                    )
                    msgs.append(m)
                return msgs

            # ================= layer 1 =================
            def l1_src(c_):
                return x_pad_d[c_ * chunk : min((c_ + 1) * chunk, npad), :]

            msgs_next = gathers(0, l1_src, 0)
            for g in range(n_groups):
                r0, r1 = g * grp, min((g + 1) * grp, nblk)
                msgs = msgs_next
                if g + 1 < n_groups:
                    msgs_next = gathers(g + 1, l1_src, 0)
                xt_g = sp2.tile([P, grp * P], bf16, tag="xt")
                nc.sync.dma_start(
                    out=xt_g[:, : (r1 - r0) * P], in_=xt_d[:, r0 * P : r1 * P]
                )
                dgb_g = sp2.tile([P, grp * P], bf16, tag="dgb")
                nc.sync.dma_start(
                    out=dgb_g[:, : (r1 - r0) * P], in_=dgb_d[:, r0 * P : r1 * P]
                )
                def dense_chain(r, meanT):
                    """hps -> relu -> zr -> zz/r2 for block r (PE-pipelined)."""
                    hps = pp.tile([P, P], f32, tag="hps")
                    nc.tensor.matmul(
                        out=hps[:, :], lhsT=w1l[:, :], rhs=meanT[:, :],
                        start=True, stop=False,
                    )
                    nc.tensor.matmul(
                        out=hps[:, :], lhsT=w1r[:, :],
                        rhs=xt_g[:, (r - r0) * P : (r - r0 + 1) * P],
                        start=False, stop=True,
                    )
                    hT = wp.tile([P, P], bf16, tag="hT")
                    nc.scalar.activation(
                        out=hT[:, :], in_=hps[:, :], func=Act.Relu, bias=b1c[:, :1]
                    )
                    zr = pp.tile([P, 2 * DO], f32, tag="zr")
                    nc.tensor.matmul(
                        out=zr[:, :], lhsT=hT[:, :], rhs=w2cat[:, :],
                        start=True, stop=True,
                    )
                    # z2 duplicated [z2|z2] bf16 -> dram for exchange
                    zz = wp.tile([P, 2 * DO], bf16, tag="zz")
                    nc.vector.tensor_copy(
                        out=zz[:, :].rearrange("p (r d) -> p r d", d=DO),
                        in_=zr[:, None, :DO].to_broadcast([P, 2, DO]),
                    )
                    nc.sync.dma_start(
                        out=z2_own_t[r * P : (r + 1) * P, :], in_=zz[:, :]
                    )
                    # r2 + b2, kept resident for layer 2
                    nc.vector.tensor_tensor(
                        out=r2_all[:, r * DO : (r + 1) * DO],
                        in0=zr[:, DO : 2 * DO],
                        in1=b2b[:, :],
                        op=Alu.add,
                    )

                pend = None
                s_next = build_sel(r0)
                for r in range(r0, r1):
                    s_t = s_next
                    aggT = pp.tile([P, P], f32, tag="agg")
                    k_total = int(TS[r])
                    kk = 0
                    for c_ in range(n_chunks):
                        for k in range(int(t_run[r][c_])):
                            scol = int(ctb[r][c_]) + k
                            mtile = int(sec_base[r][c_]) + k
                            nc.tensor.matmul(
                                out=aggT[:, :],
                                lhsT=msgs[c_][:, mtile * D : (mtile + 1) * D],
                                rhs=s_t[:, scol * P : (scol + 1) * P],
                                start=(kk == 0),
                                stop=(kk == k_total - 1),
                            )
                            kk += 1
                    if r + 1 < r1:
                        s_next = build_sel(r + 1)
                    # meanT = aggT * deginv (bcast plane), psum -> sbuf bf16
                    meanT = wp.tile([P, P], bf16, tag="meanT")
                    nc.vector.tensor_tensor(
                        out=meanT[:, :],
                        in0=aggT[:, :],
                        in1=dgb_g[:, (r - r0) * P : (r - r0 + 1) * P],
                        op=Alu.mult,
                    )
                    if pend is not None:
                        dense_chain(*pend)
                    pend = (r, meanT)
                if pend is not None:
                    dense_chain(*pend)

            # ===== exchange (deps tracked via DRAM pool tiles; no barriers) =====
            nc.gpsimd.collective_compute(
                "AllGather",
                mybir.AluOpType.bypass,
                replica_groups=[list(range(CORES))],
                ins=[z2_own_t[:, :].opt()],
                outs=[z2_full_t[:, :].opt()],
            )

            # ================= layer 2 =================
            def l2_src(c_):
                return z2_full_t[c_ * chunk : min((c_ + 1) * chunk, npad), :]

            msgs_next = gathers(0, l2_src, 0)
            for g in range(n_groups):
                r0, r1 = g * grp, min((g + 1) * grp, nblk)
                gsz = r1 - r0
                msgs = msgs_next
                if g + 1 < n_groups:
                    msgs_next = gathers(g + 1, l2_src, 0)
                # stage A per block: agg -> o -> mx -> exp(+row-sum into sms)
                sms = op8.tile([P, grp], f32, tag="sms")
                os_, mxs = [], []
                s_next = build_sel(r0)
                for r in range(r0, r1):
                    s_t = s_next
                    agg2 = pp.tile([P, DO], f32, tag="agg2")
                    k_total = int(TS[r])
                    kk = 0
                    for c_ in range(n_chunks):
                        for k in range(int(t_run[r][c_])):
                            scol = int(ctb[r][c_]) + k
                            mtile = int(sec_base[r][c_]) + k
                            nc.tensor.matmul(
                                out=agg2[:, :],
                                lhsT=s_t[:, scol * P : (scol + 1) * P],
                                rhs=msgs[c_][:, mtile * D : mtile * D + DO],
                                start=(kk == 0),
                                stop=(kk == k_total - 1),
                            )
                            kk += 1
                    if r + 1 < r1:
                        s_next = build_sel(r + 1)
                    o = op8.tile([P, DO], f32, tag="o")
                    nc.vector.tensor_scalar(
                        out=o[:, :], in0=agg2[:, :], scalar1=deginv[:, r : r + 1],
                        scalar2=None, op0=Alu.mult,
                    )
                    nc.vector.tensor_tensor(
                        out=o[:, :], in0=o[:, :],
                        in1=r2_all[:, r * DO : (r + 1) * DO], op=Alu.add,
                    )
                    mx = op8.tile([P, 1], f32, tag="mx")
                    nc.vector.reduce_max(out=mx[:, :], in_=o[:, :], axis=X)
                    nmx = op8.tile([P, 1], f32, tag="nmx")
                    nc.vector.tensor_scalar(
                        out=nmx[:, :], in0=mx[:, :], scalar1=-1.0, scalar2=None,
                        op0=Alu.mult,
                    )
                    ex = wp.tile([P, DO], f32, tag="ex")
                    nc.scalar.activation(
                        out=ex[:, :], in_=o[:, :], func=Act.Exp,
                        bias=nmx[:, :1], accum_out=sms[:, r - r0 : r - r0 + 1],
                    )
                    os_.append(o)
                    mxs.append(mx)
                # stage B: one Ln for the group, then finals
                lss = op8.tile([P, grp], f32, tag="lss")
                nc.scalar.activation(
                    out=lss[:, :gsz], in_=sms[:, :gsz], func=Act.Ln
                )
                for r in range(r0, r1):
                    j = r - r0
                    res = wp.tile([P, DO], f32, tag="res")
                    nc.vector.tensor_scalar(
                        out=res[:, :], in0=os_[j][:, :], scalar1=mxs[j][:, :1],
                        scalar2=lss[:, j : j + 1], op0=Alu.subtract, op1=Alu.subtract,
                    )
                    nc.sync.dma_start(
                        out=out_d[r * P : (r + 1) * P, :], in_=res[:, :]
                    )

    nc.finalize()
    return nc


def _run(x, edge_index, W1_l, b1, W1_r, W2_l, b2, W2_r, n_nodes, shard, trace=False):
    import ml_dtypes

    from concourse import bass_utils

    bf = ml_dtypes.bfloat16
    x = np.ascontiguousarray(np.asarray(x, dtype=np.float32))
    chunk = min(shard * CORES, 25088)
    grp = min(GRP, shard // P)
    meta = _prep(x, edge_index, n_nodes, shard, chunk, grp)
    nc = _build(meta, shard, chunk, grp)

    npad = meta["npad"]
    nblk = meta["nblk"]
    perm = meta["perm"]
    x_pad = meta["x_pad"]
    deginv = meta["deginv"]

    w2cat = np.concatenate(
        [np.asarray(W2_l, np.float32), np.asarray(W2_r, np.float32)], axis=1
    ).astype(bf)
    b1c = np.asarray(b1, np.float32).reshape(P, 1)
    b2b = np.broadcast_to(np.asarray(b2, np.float32), (P, DO)).copy()
    iota = np.broadcast_to(np.arange(P, dtype=np.float32), (P, P)).astype(bf)
    x_pad_bf = x_pad.astype(bf)
    deginv_c = deginv.reshape(CORES, nblk, P).transpose(0, 2, 1).copy()

    in_maps = []
    for c in range(CORES):
        xs = x_pad[c * shard : (c + 1) * shard]
        dg = deginv[c * shard : (c + 1) * shard]
        in_maps.append(
            {
                "x_pad": x_pad_bf,
                "xt": np.ascontiguousarray(xs.T).astype(bf),
                "dgb": np.ascontiguousarray(
                    np.broadcast_to(dg[None, :], (P, shard))
                ).astype(bf),
                "idx16": np.ascontiguousarray(meta["idx16"][c]),
                "ldst": np.ascontiguousarray(meta["ldst"][c]).astype(bf),
                "deginv": np.ascontiguousarray(deginv_c[c]),
                "w1l": np.asarray(W1_l, np.float32).astype(bf),
                "w1r": np.asarray(W1_r, np.float32).astype(bf),
                "w2cat": w2cat,
                "b1c": b1c,
                "b2b": b2b,
                "iota": iota,
            }
        )
    res = bass_utils.run_bass_kernel_spmd(
        nc, in_maps, core_ids=list(range(CORES)), trace=trace
    )
    out_new = np.concatenate([r["out"] for r in res.results], axis=0)
    out = np.empty((n_nodes, DO), np.float32)
    out[:] = out_new[perm[:n_nodes]]
    return np.ascontiguousarray(out), res


def kernel(x, edge_index, W1_l, b1, W1_r, W2_l, b2, W2_r):
    out, _ = _run(
        x, edge_index, W1_l, b1, W1_r, W2_l, b2, W2_r, n_nodes=100000, shard=12544
    )
    return out


# revision 13
# speedup vs baseline: 1.0127x; 1.0127x over previous
"""GraphSAGE (2x SAGEConv mean-aggr + log_softmax) on 8 Trainium2 NeuronCores.

v2 strategy (graph/data parallel):
  - Nodes sharded into 8 ranges of 12544; within each core, 128-node dst
    blocks are HOST-RELABELED into descending-edge-count order so the
    cross-core max tile schedule stays tight while the SPMD program is
    uniform (host un-permutes the output).
  - All features bf16 on device (4x PE, 2x DVE, ~2x gather DMA vs f32).
  - Edges routed to dst core, grouped per (dst block, 25088-row src chunk);
    per (7-block group, chunk) dma_gather with ~4-6k indices, spread over
    4 SWDGE queues (desc-gen parallelizes across Q7 core pairs).
  - Aggregation: one-hot S (DVE is_equal, bf16) x gathered messages on PE;
    layer-1 runs feature-major (aggT = msg.T @ S) so the dense chain
    h = relu(mean@W1_l + x@W1_r + b1), [z2|r2] = h@[W2_l|W2_r] needs no
    on-device transposes (weights consumed as stored).
  - deg^-1 applied via a host-precomputed broadcast plane (feature-major)
    or per-partition scalars (node-major layer 2).
  - z2 stored duplicated [z2|z2] bf16 (256 B rows: dma_gather minimum),
    exchanged with an in-kernel AllGather, then layer 2 aggregates
    node-major and finishes log_softmax with Exp(accum_out)/Ln on ACT.
"""

import sys

import numpy as np

sys.path.insert(0, "/opt/trn_rl_repo")

P = 128
D = 128
DO = 64
CORES = 8
GRP = 7
PAD_LDST = 240.0


def _prep(x, edge_index, n_nodes, shard, chunk, grp):
    npad = shard * CORES
    nblk = shard // P
    n_chunks = -(-npad // chunk)
    src = np.asarray(edge_index[0], dtype=np.int64)
    dst = np.asarray(edge_index[1], dtype=np.int64)

    # ---- host node relabel: per-core blocks sorted by edge count ----
    bc = np.bincount(dst // P, minlength=CORES * nblk).reshape(CORES, nblk)
    perm = np.empty(npad, dtype=np.int64)  # old id -> new id
    for c in range(CORES):
        order = np.argsort(-bc[c], kind="stable")  # rank -> old local block
        inv = np.empty(nblk, np.int64)
        inv[order] = np.arange(nblk)
        j = np.arange(shard)
        perm[c * shard + j] = c * shard + inv[j // P] * P + j % P
    src = perm[src]
    dst = perm[dst]
    x_pad = np.zeros((npad, D), np.float32)
    x_pad[perm[:n_nodes]] = np.asarray(x, np.float32)

    deg = np.bincount(dst, minlength=npad).astype(np.float32)
    deginv = (1.0 / np.maximum(deg, 1.0)).astype(np.float32)

    core = dst // shard
    lb = (dst % shard) // P  # rank-block
    ch = src // chunk

    cnt = np.zeros((CORES, nblk, n_chunks), np.int64)
    np.add.at(cnt, (core, lb, ch), 1)
    t_run = -(-cnt.max(axis=0) // P)  # [nblk, n_chunks]
    t_run[:, 0] = np.maximum(t_run[:, 0], 1)
    TS = t_run.sum(axis=1)  # tiles per block (S layout)
    nt = int(TS.sum())
    tmax = int(TS.max())

    # S/ldst column layout: block-major, chunks consecutive within block
    col_base_blk = np.concatenate([[0], np.cumsum(TS)])  # [nblk+1]
    ctb = np.concatenate(
        [np.zeros((nblk, 1), np.int64), np.cumsum(t_run, axis=1)[:, :-1]], axis=1
    )  # chunk tile base within block

    # group structure
    n_groups = -(-nblk // grp)
    G_T = np.zeros((n_groups, n_chunks), np.int64)  # tiles per (group, chunk)
    sec_base = np.zeros((nblk, n_chunks), np.int64)  # block tile base in section
    for g in range(n_groups):
        r0, r1 = g * grp, min((g + 1) * grp, nblk)
        for c_ in range(n_chunks):
            acc = 0
            for r in range(r0, r1):
                sec_base[r, c_] = acc
                acc += t_run[r, c_]
            G_T[g, c_] = acc
    # idx stream col bases (x8 cols per tile): gather order (g, ch)
    ig_tiles = G_T.ravel()
    idx_tile_base = np.concatenate([[0], np.cumsum(ig_tiles)]).reshape(-1)[:-1]
    idx_tile_base = idx_tile_base.reshape(n_groups, n_chunks)
    assert ig_tiles.sum() == nt

    order = np.lexsort((dst, ch, lb, core))
    src_s, dst_s, lb_s, ch_s, core_s = (
        src[order], dst[order], lb[order], ch[order], core[order]
    )

    idx16 = np.zeros((CORES, 16, nt * 8), np.int16)
    ldst = np.full((CORES, P, nt), PAD_LDST, np.float32)
    for c in range(CORES):
        m = core_s == c
        lbc, chc, d_c, s_c = lb_s[m], ch_s[m], dst_s[m], src_s[m]
        rid = lbc * n_chunks + chc
        starts = np.concatenate(
            [[0], np.cumsum(np.bincount(rid, minlength=nblk * n_chunks))]
        )
        pos = np.arange(len(d_c)) - starts[rid]
        colt = col_base_blk[lbc] + ctb[lbc, chc] + pos // P
        ldst[c, pos % P, colt] = (d_c % P).astype(np.float32)
        g = lbc // grp
        slot = (sec_base[lbc, chc] + pos // P) * P + pos % P
        icol = idx_tile_base[g, chc] * 8 + slot // 16
        idx16[c, slot % 16, icol] = (s_c - chc * chunk).astype(np.int16)
    idx16 = np.tile(idx16, (1, 8, 1))  # replicate 16 -> 128 partitions

    return dict(
        x_pad=x_pad, perm=perm, deginv=deginv, idx16=idx16, ldst=ldst,
        t_run=t_run, TS=TS, nt=nt, tmax=tmax, nblk=nblk, npad=npad,
        n_chunks=n_chunks, n_groups=n_groups, col_base_blk=col_base_blk,
        ctb=ctb, G_T=G_T, sec_base=sec_base, idx_tile_base=idx_tile_base,
    )


def _build(meta, shard, chunk, grp):
    import concourse.mybir as mybir
    import concourse.tile as tile
    from concourse.bacc import Bacc

    f32 = mybir.dt.float32
    bf16 = mybir.dt.bfloat16
    i16 = mybir.dt.int16
    Alu = mybir.AluOpType
    Act = mybir.ActivationFunctionType
    X = mybir.AxisListType.X

    nblk = meta["nblk"]
    npad = meta["npad"]
    nt = meta["nt"]
    tmax = meta["tmax"]
    n_chunks = meta["n_chunks"]
    n_groups = meta["n_groups"]
    t_run = meta["t_run"]
    col_base_blk = meta["col_base_blk"]
    ctb = meta["ctb"]
    G_T = meta["G_T"]
    sec_base = meta["sec_base"]
    idx_tile_base = meta["idx_tile_base"]
    TS = meta["TS"]
    GTmax = [int(G_T[:, c_].max()) for c_ in range(n_chunks)]

    nc = Bacc(num_swdge_queues=4)
    # ---- kernel I/O ----
    x_pad_d = nc.dram_tensor("x_pad", [npad, D], bf16, kind="ExternalInput")
    xt_d = nc.dram_tensor("xt", [P, shard], bf16, kind="ExternalInput")
    dgb_d = nc.dram_tensor("dgb", [P, shard], bf16, kind="ExternalInput")
    idx_d = nc.dram_tensor("idx16", [P, nt * 8], i16, kind="ExternalInput")
    ldst_d = nc.dram_tensor("ldst", [P, nt], bf16, kind="ExternalInput")
    deginv_d = nc.dram_tensor("deginv", [P, nblk], f32, kind="ExternalInput")
    w1l_d = nc.dram_tensor("w1l", [D, D], bf16, kind="ExternalInput")
    w1r_d = nc.dram_tensor("w1r", [D, D], bf16, kind="ExternalInput")
    w2cat_d = nc.dram_tensor("w2cat", [D, 2 * DO], bf16, kind="ExternalInput")
    b1c_d = nc.dram_tensor("b1c", [P, 1], f32, kind="ExternalInput")
    b2b_d = nc.dram_tensor("b2b", [P, DO], f32, kind="ExternalInput")
    iota_d = nc.dram_tensor("iota", [P, P], bf16, kind="ExternalInput")
    out_d = nc.dram_tensor("out", [shard, DO], f32, kind="ExternalOutput")
    z2_full_d = nc.dram_tensor(
        "z2_full", [npad, 2 * DO], bf16, kind="Internal", addr_space="Shared"
    )

    with tile.TileContext(nc) as tc:
        with (
            tc.tile_pool(name="const", bufs=1) as cp,
            tc.tile_pool(name="stream", bufs=2) as sp2,
            tc.tile_pool(name="msg", bufs=2) as mp,
            tc.tile_pool(name="sel", bufs=2) as sep,
            tc.tile_pool(name="work", bufs=3) as wp,
            tc.tile_pool(name="pipe", bufs=grp + 1) as op8,
            tc.tile_pool(name="dram", bufs=1, space="DRAM") as dp,
            tc.tile_pool(name="psum", bufs=2, space="PSUM") as pp,
        ):
            z2_own_t = dp.tile([shard, 2 * DO], bf16)
            w1l = cp.tile_from(w1l_d[:, :])
            w1r = cp.tile_from(w1r_d[:, :])
            w2cat = cp.tile_from(w2cat_d[:, :])
            b1c = cp.tile_from(b1c_d[:, :])
            b2b = cp.tile_from(b2b_d[:, :])
            iota = cp.tile_from(iota_d[:, :])
            deginv = cp.tile_from(deginv_d[:, :])
            idx = cp.tile_from(idx_d[:, :])
            ldst = cp.tile_from(ldst_d[:, :])
            r2_all = cp.tile([P, nblk * DO], f32)

            def build_sel(r):
                t = int(TS[r])
                c0 = int(col_base_blk[r])
                s_t = sep.tile([P, tmax * P], bf16, tag="S")
                l0 = ldst[:, c0 : c0 + t][:, :, None].to_broadcast([P, t, P])
                i0 = iota[:, :][:, None, :].to_broadcast([P, t, P])
                nc.vector.tensor_tensor(
                    out=s_t[:, : t * P].rearrange("p (t s) -> p t s", s=P),
                    in0=l0,
                    in1=i0,
                    op=Alu.is_equal,
                )
                return s_t

            def gathers(g, src_view, qoff):
                """Launch the n_chunks gathers for group g; returns msg tiles."""
                msgs = []
                for c_ in range(n_chunks):
                    gt = int(G_T[g][c_])
                    m = mp.tile([P, GTmax[c_] * D], bf16, tag=f"m{c_}")
                    nidx = gt * P
                    nc.gpsimd.dma_gather(
                        out_ap=m[:, : gt * D].rearrange("p (t d) -> p t d", d=D),
                        in_ap=src_view(c_),
                        idxs_ap=idx[
                            :,
                            int(idx_tile_base[g][c_]) * 8 : (
                                int(idx_tile_base[g][c_]) + gt
                            )
                            * 8,
                        ],
                        num_idxs=nidx,
                        num_idxs_reg=nidx,
                        elem_size=D,
                        elem_step=D,
                        single_packet=False,
                        queue_num=(c_ + qoff) % n_chunks if n_chunks >= 4 else (g + c_ + qoff) % 4,
                    )
                    msgs.append(m)
                return msgs

            # ================= layer 1 =================
            def l1_src(c_):
                return x_pad_d[c_ * chunk : min((c_ + 1) * chunk, npad), :]

            msgs_next = gathers(0, l1_src, 0)
            for g in range(n_groups):
                r0, r1 = g * grp, min((g + 1) * grp, nblk)
                msgs = msgs_next
                if g + 1 < n_groups:
                    msgs_next = gathers(g + 1, l1_src, 0)
                xt_g = sp2.tile([P, grp * P], bf16, tag="xt")
                nc.sync.dma_start(
                    out=xt_g[:, : (r1 - r0) * P], in_=xt_d[:, r0 * P : r1 * P]
                )
                dgb_g = sp2.tile([P, grp * P], bf16, tag="dgb")
                nc.sync.dma_start(
                    out=dgb_g[:, : (r1 - r0) * P], in_=dgb_d[:, r0 * P : r1 * P]
                )
                def dense_chain(r, meanT):
                    """hps -> relu -> zr -> zz/r2 for block r (PE-pipelined)."""
                    hps = pp.tile([P, P], f32, tag="hps")
                    nc.tensor.matmul(
                        out=hps[:, :], lhsT=w1l[:, :], rhs=meanT[:, :],
                        start=True, stop=False,
                    )
                    nc.tensor.matmul(
                        out=hps[:, :], lhsT=w1r[:, :],
                        rhs=xt_g[:, (r - r0) * P : (r - r0 + 1) * P],
                        start=False, stop=True,
                    )
                    hT = wp.tile([P, P], bf16, tag="hT")
                    nc.scalar.activation(
                        out=hT[:, :], in_=hps[:, :], func=Act.Relu, bias=b1c[:, :1]
                    )
                    zr = pp.tile([P, 2 * DO], f32, tag="zr")
                    nc.tensor.matmul(
                        out=zr[:, :], lhsT=hT[:, :], rhs=w2cat[:, :],
                        start=True, stop=True,
                    )
                    # z2 duplicated [z2|z2] bf16 -> dram for exchange
                    zz = wp.tile([P, 2 * DO], bf16, tag="zz")
                    nc.vector.tensor_copy(
                        out=zz[:, :].rearrange("p (r d) -> p r d", d=DO),
                        in_=zr[:, None, :DO].to_broadcast([P, 2, DO]),
                    )
                    nc.sync.dma_start(
                        out=z2_own_t[r * P : (r + 1) * P, :], in_=zz[:, :]
                    )
                    # r2 + b2, kept resident for layer 2
                    nc.vector.tensor_tensor(
                        out=r2_all[:, r * DO : (r + 1) * DO],
                        in0=zr[:, DO : 2 * DO],
                        in1=b2b[:, :],
                        op=Alu.add,
                    )

                pend = None
                s_next = build_sel(r0)
                for r in range(r0, r1):
                    s_t = s_next
                    aggT = pp.tile([P, P], f32, tag="agg")
                    k_total = int(TS[r])
                    kk = 0
                    for c_ in range(n_chunks):
                        for k in range(int(t_run[r][c_])):
                            scol = int(ctb[r][c_]) + k
                            mtile = int(sec_base[r][c_]) + k
                            nc.tensor.matmul(
                                out=aggT[:, :],
                                lhsT=msgs[c_][:, mtile * D : (mtile + 1) * D],
                                rhs=s_t[:, scol * P : (scol + 1) * P],
                                start=(kk == 0),
                                stop=(kk == k_total - 1),
                            )
                            kk += 1
                    if r + 1 < r1:
                        s_next = build_sel(r + 1)
                    # meanT = aggT * deginv (bcast plane), psum -> sbuf bf16
                    meanT = wp.tile([P, P], bf16, tag="meanT")
                    nc.vector.tensor_tensor(
                        out=meanT[:, :],
                        in0=aggT[:, :],
                        in1=dgb_g[:, (r - r0) * P : (r - r0 + 1) * P],
                        op=Alu.mult,
                    )
                    if pend is not None:
                        dense_chain(*pend)
                    pend = (r, meanT)
                if pend is not None:
                    dense_chain(*pend)

            # ===== exchange (deps tracked via DRAM pool tiles; no barriers) =====
            nc.gpsimd.collective_compute(
                "AllGather",
                mybir.AluOpType.bypass,
                replica_groups=[list(range(CORES))],
                ins=[z2_own_t[:, :].opt()],
                outs=[z2_full_d[:, :]],
            )
            tc.strict_bb_all_engine_barrier()

            # ================= layer 2 =================
            def l2_src(c_):
                return z2_full_d[c_ * chunk : min((c_ + 1) * chunk, npad), :]

            msgs_next = gathers(0, l2_src, 0)
            for g in range(n_groups):
                r0, r1 = g * grp, min((g + 1) * grp, nblk)
                gsz = r1 - r0
                msgs = msgs_next
                if g + 1 < n_groups:
                    msgs_next = gathers(g + 1, l2_src, 0)
                # stage A per block: agg -> o -> mx -> exp(+row-sum into sms)
                sms = op8.tile([P, grp], f32, tag="sms")
                os_, mxs = [], []
                s_next = build_sel(r0)
                for r in range(r0, r1):
                    s_t = s_next
                    agg2 = pp.tile([P, DO], f32, tag="agg2")
                    k_total = int(TS[r])
                    kk = 0
                    for c_ in range(n_chunks):
                        for k in range(int(t_run[r][c_])):
                            scol = int(ctb[r][c_]) + k
                            mtile = int(sec_base[r][c_]) + k
                            nc.tensor.matmul(
                                out=agg2[:, :],
                                lhsT=s_t[:, scol * P : (scol + 1) * P],
                                rhs=msgs[c_][:, mtile * D : mtile * D + DO],
                                start=(kk == 0),
                                stop=(kk == k_total - 1),
                            )
                            kk += 1
                    if r + 1 < r1:
                        s_next = build_sel(r + 1)
                    o = op8.tile([P, DO], f32, tag="o")
                    nc.vector.tensor_scalar(
                        out=o[:, :], in0=agg2[:, :], scalar1=deginv[:, r : r + 1],
                        scalar2=None, op0=Alu.mult,
                    )
                    nc.vector.tensor_tensor(
                        out=o[:, :], in0=o[:, :],
                        in1=r2_all[:, r * DO : (r + 1) * DO], op=Alu.add,
                    )
                    mx = op8.tile([P, 1], f32, tag="mx")
                    nc.vector.reduce_max(out=mx[:, :], in_=o[:, :], axis=X)
                    nmx = op8.tile([P, 1], f32, tag="nmx")
                    nc.vector.tensor_scalar(
                        out=nmx[:, :], in0=mx[:, :], scalar1=-1.0, scalar2=None,
                        op0=Alu.mult,
                    )
                    ex = wp.tile([P, DO], f32, tag="ex")
                    nc.scalar.activation(
                        out=ex[:, :], in_=o[:, :], func=Act.Exp,
                        bias=nmx[:, :1], accum_out=sms[:, r - r0 : r - r0 + 1],
                    )
                    os_.append(o)
                    mxs.append(mx)
                # stage B: one Ln for the group, then finals
                lss = op8.tile([P, grp], f32, tag="lss")
                nc.scalar.activation(
                    out=lss[:, :gsz], in_=sms[:, :gsz], func=Act.Ln
                )
                for r in range(r0, r1):
                    j = r - r0
                    res = wp.tile([P, DO], f32, tag="res")
                    nc.vector.tensor_scalar(
                        out=res[:, :], in0=os_[j][:, :], scalar1=mxs[j][:, :1],
                        scalar2=lss[:, j : j + 1], op0=Alu.subtract, op1=Alu.subtract,
                    )
                    nc.sync.dma_start(
                        out=out_d[r * P : (r + 1) * P, :], in_=res[:, :]
                    )

    nc.finalize()
    return nc


def _run(x, edge_index, W1_l, b1, W1_r, W2_l, b2, W2_r, n_nodes, shard, trace=False):
    import ml_dtypes

    from concourse import bass_utils

    bf = ml_dtypes.bfloat16
    x = np.ascontiguousarray(np.asarray(x, dtype=np.float32))
    chunk = min(shard * CORES, 25088)
    grp = min(GRP, shard // P)
    meta = _prep(x, edge_index, n_nodes, shard, chunk, grp)
    nc = _build(meta, shard, chunk, grp)

    npad = meta["npad"]
    nblk = meta["nblk"]
    perm = meta["perm"]
    x_pad = meta["x_pad"]
    deginv = meta["deginv"]

    w2cat = np.concatenate(
        [np.asarray(W2_l, np.float32), np.asarray(W2_r, np.float32)], axis=1
    ).astype(bf)
    b1c = np.asarray(b1, np.float32).reshape(P, 1)
    b2b = np.broadcast_to(np.asarray(b2, np.float32), (P, DO)).copy()
    iota = np.broadcast_to(np.arange(P, dtype=np.float32), (P, P)).astype(bf)
    x_pad_bf = x_pad.astype(bf)
    deginv_c = deginv.reshape(CORES, nblk, P).transpose(0, 2, 1).copy()

    in_maps = []
    for c in range(CORES):
        xs = x_pad[c * shard : (c + 1) * shard]
        dg = deginv[c * shard : (c + 1) * shard]
        in_maps.append(
            {
                "x_pad": x_pad_bf,
                "xt": np.ascontiguousarray(xs.T).astype(bf),
                "dgb": np.ascontiguousarray(
                    np.broadcast_to(dg[None, :], (P, shard))
                ).astype(bf),
                "idx16": np.ascontiguousarray(meta["idx16"][c]),
                "ldst": np.ascontiguousarray(meta["ldst"][c]).astype(bf),
                "deginv": np.ascontiguousarray(deginv_c[c]),
                "w1l": np.asarray(W1_l, np.float32).astype(bf),
                "w1r": np.asarray(W1_r, np.float32).astype(bf),
                "w2cat": w2cat,
                "b1c": b1c,
                "b2b": b2b,
                "iota": iota,
            }
        )
    res = bass_utils.run_bass_kernel_spmd(
        nc, in_maps, core_ids=list(range(CORES)), trace=trace
    )
    out_new = np.concatenate([r["out"] for r in res.results], axis=0)
    out = np.empty((n_nodes, DO), np.float32)
    out[:] = out_new[perm[:n_nodes]]
    return np.ascontiguousarray(out), res


def kernel(x, edge_index, W1_l, b1, W1_r, W2_l, b2, W2_r):
    out, _ = _run(
        x, edge_index, W1_l, b1, W1_r, W2_l, b2, W2_r, n_nodes=100000, shard=12544
    )
    return out
